# revision 1
# baseline (speedup 1.0000x reference)
"""Fused 2D-RoPE multi-head attention block for Trainium2, SPMD over 8 NeuronCores.

Problem: x[2,4,24,24,1024] -> qkv proj -> 16-head attention with 2-axis RoPE
-> out proj.  Data-parallel: the fused (b t) dim has 8 sequences; one
sequence (S=576 tokens, D=1024) per core.

Device-side layout choices (everything picked so no on-device transposes are
needed):
  - x is fed pre-transposed per core: xT [D, S].
  - q,k are produced in [e, s] layout (head-dim on partitions) by using the
    (host-pre-transposed) weight as the stationary operand.
  - Within each head, q/k weight rows are host-permuted to even-pairs-first
    order so the RoPE rotate-half pair swap becomes a contiguous
    32-partition block swap (plain DMAs; strided-partition DMA is broken).
  - v is produced in natural [s, e] layout (x as stationary operand), padded
    with a ones-column per head (65-wide slots) so the softmax denominator
    falls out of the same matmul that computes att@v.
  - Attention is computed as scoresT[sk, sq] = k_ropedT-stationary x
    q_ropedT, exp on ScalarE (no max subtraction: scores ~ N(0,1), exp is
    safe), then oT[dh, sq] = v_aug-stationary x E, which leaves oT in
    exactly the [d, s] layout the output projection needs as its stationary
    operand.
  - All matmuls run as float32r (TF32-like precision, ~10x better than
    bf16; measured end-to-end rel err 3.8e-4 vs the fp32 reference). The
    projections stream 256/288-wide moving chunks, which hit a fast PE
    streaming mode measured at ~25-60ns per 128x128 accumulation step.

Outputs of the 8 cores are gathered and reshaped on the host; b_out is added
on the host (it is all-zeros in the reference inputs anyway).
"""

import numpy as np
from contextlib import ExitStack

B, T, HH, WW, D = 2, 4, 24, 24, 1024
NH, HD = 16, 64
S = HH * WW            # 576
BT = B * T             # 8
NCORES = 8
P = 128
SQH = 288              # half of S; moving-dim per scores/att@v matmul
NKD = D // P           # 8 contraction tiles over D
S_TILES = [(0, 128), (128, 128), (256, 128), (384, 128), (512, 64)]
VSLOT = HD + 1         # 65: per-head v columns + ones column

_CACHE: dict = {}


def _rope_tables():
    """cos/sin tables in the permuted (evens-first) [128, S] block layout."""
    half = HD // 4     # 16
    inv = (1.0 / (10000.0 ** (np.arange(half, dtype=np.float32) / np.float32(half)))).astype(np.float32)
    th = np.arange(HH, dtype=np.float32)[:, None] * inv[None, :]          # [H, 16]
    tw = np.arange(WW, dtype=np.float32)[:, None] * inv[None, :]          # [W, 16]
    cosg = np.concatenate([
        np.broadcast_to(np.cos(th)[:, None, :], (HH, WW, half)),
        np.broadcast_to(np.cos(tw)[None, :, :], (HH, WW, half))], axis=-1).reshape(S, 2 * half)
    sing = np.concatenate([
        np.broadcast_to(np.sin(th)[:, None, :], (HH, WW, half)),
        np.broadcast_to(np.sin(tw)[None, :, :], (HH, WW, half))], axis=-1).reshape(S, 2 * half)
    cosb = np.concatenate([cosg, cosg], axis=1).T          # [64, S]
    sinb = np.concatenate([sing, -sing], axis=1).T         # [64, S] (pre-swapped)
    cosb = np.ascontiguousarray(np.vstack([cosb, cosb]).astype(np.float32))   # [128, S]
    sinb = np.ascontiguousarray(np.vstack([sinb, sinb]).astype(np.float32))
    return cosb, sinb


def _head_perm():
    """Permutation of w_qkv q/k rows: within each head, evens then odds."""
    perm64 = np.concatenate([np.arange(0, HD, 2), np.arange(1, HD, 2)])
    return (np.arange(NH)[:, None] * HD + perm64[None, :]).reshape(-1)     # [1024]


def _build_nc(repeat=1):
    import concourse.bacc as bacc
    import concourse.mybir as mybir
    from concourse.tile import TileContext

    f32 = mybir.dt.float32
    f32r = mybir.dt.float32r
    AF = mybir.ActivationFunctionType

    nc = bacc.Bacc("TRN2", target_bir_lowering=False, debug=False)
    xT_d = nc.dram_tensor("xT", [D, S], f32r, kind="ExternalInput").ap()
    wqk_d = nc.dram_tensor("wqkT", [D, 2 * D], f32r, kind="ExternalInput").ap()
    wv_d = nc.dram_tensor("wvT", [D, D], f32r, kind="ExternalInput").ap()
    wo_d = nc.dram_tensor("woT", [D, D], f32r, kind="ExternalInput").ap()
    cos_d = nc.dram_tensor("cosb", [P, S], f32, kind="ExternalInput").ap()
    sin_d = nc.dram_tensor("sinb", [P, S], f32, kind="ExternalInput").ap()
    ones_d = nc.dram_tensor("onesc", [P, 5 * NH], f32r, kind="ExternalInput").ap()
    out_d = nc.dram_tensor("out", [S, D], f32, kind="ExternalOutput").ap()

    with TileContext(nc) as tc, ExitStack() as ctx:
        const = ctx.enter_context(tc.tile_pool(name="const", bufs=1))
        wqkp = ctx.enter_context(tc.tile_pool(name="wqkp", bufs=3))
        wvp = ctx.enter_context(tc.tile_pool(name="wvp", bufs=3))
        wop = ctx.enter_context(tc.tile_pool(name="wop", bufs=3))
        rawp = ctx.enter_context(tc.tile_pool(name="rawp", bufs=3))
        m2p = ctx.enter_context(tc.tile_pool(name="m2p", bufs=2))
        ep = ctx.enter_context(tc.tile_pool(name="ep", bufs=12))
        r1p = ctx.enter_context(tc.tile_pool(name="r1p", bufs=4))
        rrp = ctx.enter_context(tc.tile_pool(name="rrp", bufs=4))
        stp = ctx.enter_context(tc.tile_pool(name="stp", bufs=3))
        psum = ctx.enter_context(tc.tile_pool(name="psum", bufs=8, space="PSUM"))

        # ---- resident tensors
        xt = const.tile([P, NKD * S], f32r, name="xt")
        cosb = const.tile([P, S], f32, name="cosb_t")
        sinb = const.tile([P, S], f32, name="sinb_t")
        roped = const.tile([P, 2 * NH * S], f32r, name="roped")    # 16 e-tiles (q then k)
        va = const.tile([P, 5 * NH * VSLOT], f32r, name="va")      # v, 65-wide head slots
        vav = va.rearrange("p (j h c) -> p j h c", j=5, c=VSLOT)
        oT = const.tile([P, NKD * S], f32r, name="oT")

        for _rep in range(repeat):
            # ---- q,k projection (+ RoPE) for one e-tile, pipelined per head-pair
            def emit_qk(et):
                ps0 = psum.tile([P, SQH], f32, tag="ps", name="ps_qk0")
                ps1 = psum.tile([P, SQH], f32, tag="ps", name="ps_qk1")
                wt = wqkp.tile([P, NKD * P], f32r, name="wt")
                nc.sync.dma_start(wt.rearrange("p (kt c) -> p kt c", c=P),
                                  wqk_d[:, et * P:(et + 1) * P].rearrange("(kt p) c -> p kt c", p=P))
                for kt in range(NKD):
                    w_r = wt[:, kt * P:(kt + 1) * P]
                    nc.tensor.matmul(ps0[:, :], w_r, xt[:, kt * S:kt * S + SQH],
                                     start=(kt == 0), stop=(kt == NKD - 1))
                    nc.tensor.matmul(ps1[:, :], w_r, xt[:, kt * S + SQH:kt * S + S],
                                     start=(kt == 0), stop=(kt == NKD - 1))
                raw = rawp.tile([P, S], f32, name="raw")
                nc.scalar.activation(raw[:, 0:SQH], ps0[:, :], AF.Copy)
                nc.scalar.activation(raw[:, SQH:S], ps1[:, :], AF.Copy)
                m2 = m2p.tile([P, S], f32, name="m2")
                for b0 in range(0, P, 64):
                    nc.vector.tensor_mul(m2[b0 + 32:b0 + 64, :], raw[b0:b0 + 32, :], sinb[b0:b0 + 32, :])
                    nc.gpsimd.tensor_mul(m2[b0:b0 + 32, :], raw[b0 + 32:b0 + 64, :], sinb[b0 + 32:b0 + 64, :])
                rsl = roped[:, et * S:(et + 1) * S]
                nc.vector.tensor_mul(rsl, raw[:, :], cosb[:, :])
                nc.vector.tensor_add(rsl, rsl, m2[:, :])

            # ---- attention for a head pair (both sq halves); the two heads sit on
            # disjoint PE row-groups (partitions 0:64 / 64:128), so interleaving
            # their scoresT matmuls lets them run concurrently in the array.
            def emit_att_pair(ti):
                qb = ti * S
                kb = (8 + ti) * S
                for hf in range(2):
                    col = slice(qb + hf * SQH, qb + (hf + 1) * SQH)
                    Es = {0: [], 1: []}
                    for j, (k0, kl) in enumerate(S_TILES):
                        pss = {}
                        for sub in range(2):
                            off = sub * 64
                            ps_s = psum.tile([P, SQH], f32, tag="ps", name="ps_s")
                            nc.tensor.matmul(ps_s[0:kl, :],
                                             roped[off:off + 64, kb + k0:kb + k0 + kl],
                                             roped[off:off + 64, col],
                                             start=True, stop=True)
                            pss[sub] = ps_s
                        for sub in range(2):
                            E = ep.tile([P, SQH], f32r, name="E")
                            nc.scalar.activation(E[0:kl, :], pss[sub][0:kl, :], AF.Exp, scale=0.125)
                            Es[sub].append(E)
                    for sub in range(2):
                        h = 2 * ti + sub
                        off = sub * 64
                        ps_o = psum.tile([P, SQH], f32, tag="ps", name="ps_o")
                        for j, (k0, kl) in enumerate(S_TILES):
                            nc.tensor.matmul(ps_o[0:VSLOT, :],
                                             vav[0:kl, j:j + 1, h:h + 1, :],
                                             Es[sub][j][0:kl, :],
                                             start=(j == 0), stop=(j == 4))
                        r1 = r1p.tile([1, SQH], f32, name="r1")
                        nc.vector.reciprocal(r1[:, :], ps_o[HD:HD + 1, :])
                        rr = rrp.tile([64, SQH], f32, name="rr")
                        nc.gpsimd.partition_broadcast(rr[:, :], r1[:, :])
                        nc.vector.tensor_mul(
                            oT[off:off + 64, ti * S + hf * SQH:ti * S + (hf + 1) * SQH],
                            ps_o[0:HD, :], rr[:, :])

            # ---- v projection first, then pair-pipelined qk+attention
            for nhf in range(2):
                for c in range(2):
                    psv = [psum.tile([P, 256], f32, tag="ps", name=f"ps_v{st}") for st in range(5)]
                    for kt2 in range(NKD // 2):
                        if nhf == 0 and c == 0:
                            for i in (2 * kt2, 2 * kt2 + 1):
                                nc.sync.dma_start(xt[:, i * S:(i + 1) * S], xT_d[i * P:(i + 1) * P, :])
                        wvt = wvp.tile([P, 512], f32r, name="wvt")
                        nc.sync.dma_start(wvt.rearrange("p (two cc) -> p two cc", cc=256),
                                          wv_d[kt2 * 2 * P:(kt2 * 2 + 2) * P,
                                               nhf * 512 + c * 256:nhf * 512 + (c + 1) * 256]
                                          .rearrange("(two p) cc -> p two cc", p=P))
                        for j in range(2):
                            kt = kt2 * 2 + j
                            for st, (s0, sl) in enumerate(S_TILES):
                                nc.tensor.matmul(psv[st][0:sl, :],
                                                 xt[:, kt * S + s0:kt * S + s0 + sl],
                                                 wvt[:, j * 256:(j + 1) * 256],
                                                 start=(kt == 0), stop=(kt == NKD - 1))
                    for st, (s0, sl) in enumerate(S_TILES):
                        dst = vav[0:sl, st:st + 1, nhf * 8 + c * 4:nhf * 8 + (c + 1) * 4, 0:HD]
                        vsrc = psv[st][0:sl, :].rearrange("p (h cc) -> p h cc", cc=HD)
                        nc.vector.tensor_copy(dst, vsrc)

            nc.sync.dma_start(cosb[:, :], cos_d[:, :])
            nc.sync.dma_start(sinb[:, :], sin_d[:, :])
            nc.sync.dma_start(va.rearrange("p (g c) -> p g c", c=VSLOT)[:, :, HD:HD + 1],
                              ones_d[:, :])
            for pr in range(8):
                emit_qk(pr)
                emit_qk(8 + pr)
            for ti in range(8):
                emit_att_pair(ti)

            # ---- output projection : out[s, e] = oT-tiles.T @ woT
            for nhf in range(2):
                for c in range(2):
                    pso = [psum.tile([P, 256], f32, tag="ps", name=f"ps_o{st}") for st in range(5)]
                    for kt2 in range(NKD // 2):
                        wot = wop.tile([P, 512], f32r, name="wot")
                        nc.sync.dma_start(wot.rearrange("p (two cc) -> p two cc", cc=256),
                                          wo_d[kt2 * 2 * P:(kt2 * 2 + 2) * P,
                                               nhf * 512 + c * 256:nhf * 512 + (c + 1) * 256]
                                          .rearrange("(two p) cc -> p two cc", p=P))
                        for j in range(2):
                            kt = kt2 * 2 + j
                            for st, (s0, sl) in enumerate(S_TILES):
                                nc.tensor.matmul(pso[st][0:sl, :],
                                                 oT[:, kt * S + s0:kt * S + s0 + sl],
                                                 wot[:, j * 256:(j + 1) * 256],
                                                 start=(kt == 0), stop=(kt == NKD - 1))
                    for st, (s0, sl) in enumerate(S_TILES):
                        stg = stp.tile([P, 256], f32, name="stg")
                        nc.vector.tensor_copy(stg[0:sl, :], pso[st][0:sl, :])
                        nc.sync.dma_start(out_d[s0:s0 + sl, nhf * 512 + c * 256:nhf * 512 + (c + 1) * 256],
                                          stg[0:sl, :])
    nc.compile()
    return nc


def _prep_inputs(x, w_qkv, w_out):
    x = np.asarray(x, dtype=np.float32)
    w_qkv = np.asarray(w_qkv, dtype=np.float32)
    w_out = np.asarray(w_out, dtype=np.float32)
    xr = x.reshape(BT, S, D)
    perm = _head_perm()
    wq = w_qkv[0:D][perm]
    wk = w_qkv[D:2 * D][perm]
    wqkT = np.ascontiguousarray(np.concatenate([wq, wk], axis=0).T)
    wvT = np.ascontiguousarray(w_qkv[2 * D:3 * D].T)
    woT = np.ascontiguousarray(w_out.T)
    cosb, sinb = _rope_tables()
    in_maps = []
    for i in range(NCORES):
        in_maps.append({
            "xT": np.ascontiguousarray(xr[i].T),
            "wqkT": wqkT, "wvT": wvT, "woT": woT,
            "cosb": cosb, "sinb": sinb,
            "onesc": np.ones((P, 5 * NH), dtype=np.float32),
        })
    return in_maps


def get_nc(repeat=1):
    key = f"nc{repeat}"
    if key not in _CACHE:
        _CACHE[key] = _build_nc(repeat)
    return _CACHE[key]


def kernel(x, w_qkv, w_out, b_out):
    from concourse import bass_utils
    nc = get_nc()
    in_maps = _prep_inputs(x, w_qkv, w_out)
    res = bass_utils.run_bass_kernel_spmd(nc, in_maps, core_ids=list(range(NCORES)))
    out = np.stack([res.results[i]["out"] for i in range(NCORES)], axis=0)
    out = out + np.asarray(b_out, dtype=np.float32)[None, None, :]
    return np.ascontiguousarray(out.reshape(B, T, HH, WW, D).astype(np.float32))



# revision 3
# speedup vs baseline: 1.4561x; 1.4561x over previous
"""Fused 2D-RoPE multi-head attention block for Trainium2, SPMD over 8 cores.

Per core: one sequence (S=576 tokens, D=1024), x -> qkv -> 16-head attention
with 2-axis RoPE -> out projection.  All matmul operands are bf16 (PSUM
accumulation stays fp32; measured end-to-end rel err 6.9e-3), which enables
the PE fast-weight-load path and halves DVE/DMA traffic.

Structure (per repeat iteration i), software-pipelined emission:
  - ATT_i: per head h, the scores -> exp -> att@v chain INTERLEAVED with the
    QK projection matmuls of iteration i+1 (e-tile h) and group 0's
    normalization.  The ScalarE exp is the attention bottleneck; the
    interleaved QK matmuls fill the PE gaps so the PE HAM clock gate stays
    at 2.4GHz instead of re-throttling to 1.2.
  - Scores for one (head, sk-tile) land in one [128,1024] PSUM tile (both
    sq-halves at bank offsets 0/512); one strided ScalarE Exp covers both.
  - Softmax denominators (ones-column of the padded v tiles) are gathered
    across partitions with one SBUF->SBUF DMA per 8-head group,
    reciprocal'd in ONE batched DVE op (replacing 32 serial [1,288]
    reciprocals at 2.3us each), broadcast 1->64 partitions via a tiny
    selector-stationary matmul, and applied by DVE muls.
  - V_{i+1} then group 1's normalization tail, then OUT_i: the V matmuls
    cover the reciprocal chain and the out-proj weight DMAs.  V/OUT use
    512-wide moving chunks (full PSUM bank per matmul).
  - x and roped q/k are double/triple-buffered; x for iteration i+2 is
    prefetched during iteration i so the interleaved QK never stalls.
"""

import numpy as np
from contextlib import ExitStack

B, T, HH, WW, D = 2, 4, 24, 24, 1024
NH, HD = 16, 64
S = HH * WW            # 576
BT = B * T             # 8
NCORES = 8
P = 128
CH = 288               # half of S; sq chunk per scores/att@v matmul
NKD = D // P           # 8 contraction tiles over D
S_TILES = [(0, 128), (128, 128), (256, 128), (384, 128), (512, 64)]

_CACHE: dict = {}


def _rope_tables():
    """cos/sin tables in the permuted (evens-first) [128, S] block layout."""
    half = HD // 4     # 16
    inv = (1.0 / (10000.0 ** (np.arange(half, dtype=np.float32) / np.float32(half)))).astype(np.float32)
    th = np.arange(HH, dtype=np.float32)[:, None] * inv[None, :]          # [H, 16]
    tw = np.arange(WW, dtype=np.float32)[:, None] * inv[None, :]          # [W, 16]
    cosg = np.concatenate([
        np.broadcast_to(np.cos(th)[:, None, :], (HH, WW, half)),
        np.broadcast_to(np.cos(tw)[None, :, :], (HH, WW, half))], axis=-1).reshape(S, 2 * half)
    sing = np.concatenate([
        np.broadcast_to(np.sin(th)[:, None, :], (HH, WW, half)),
        np.broadcast_to(np.sin(tw)[None, :, :], (HH, WW, half))], axis=-1).reshape(S, 2 * half)
    cosb = np.concatenate([cosg, cosg], axis=1).T          # [64, S]
    sinb = np.concatenate([sing, -sing], axis=1).T         # [64, S] (pre-swapped)
    cosb = np.ascontiguousarray(np.vstack([cosb, cosb]).astype(np.float32))   # [128, S]
    sinb = np.ascontiguousarray(np.vstack([sinb, sinb]).astype(np.float32))
    return cosb, sinb


def _head_perm():
    """Permutation of w_qkv q/k rows: within each head, evens then odds."""
    perm64 = np.concatenate([np.arange(0, HD, 2), np.arange(1, HD, 2)])
    return (np.arange(NH)[:, None] * HD + perm64[None, :]).reshape(-1)     # [1024]


def _build_nc(repeat=1):
    import concourse.bacc as bacc
    import concourse.mybir as mybir
    from concourse.tile import TileContext

    f32 = mybir.dt.float32
    f32r = mybir.dt.float32r
    bf16 = mybir.dt.bfloat16
    AF = mybir.ActivationFunctionType

    nc = bacc.Bacc("TRN2", target_bir_lowering=False, debug=False)
    xT_d = nc.dram_tensor("xT", [D, S], bf16, kind="ExternalInput").ap()
    wqk_d = nc.dram_tensor("wqkT", [D, 2 * D], bf16, kind="ExternalInput").ap()
    wv_d = nc.dram_tensor("wvT", [D, D], bf16, kind="ExternalInput").ap()
    wo_d = nc.dram_tensor("woT", [D, D], bf16, kind="ExternalInput").ap()
    cos_d = nc.dram_tensor("cosb", [P, S], bf16, kind="ExternalInput").ap()
    sin_d = nc.dram_tensor("sinb", [P, S], bf16, kind="ExternalInput").ap()
    sel_d = nc.dram_tensor("selc", [64, NH * HD], f32r, kind="ExternalInput").ap()
    out_d = nc.dram_tensor("out", [S, D], f32, kind="ExternalOutput").ap()

    with TileContext(nc) as tc, ExitStack() as ctx:
        const = ctx.enter_context(tc.tile_pool(name="const", bufs=1))
        xtp = ctx.enter_context(tc.tile_pool(name="xtp", bufs=3))
        ropedp = ctx.enter_context(tc.tile_pool(name="ropedp", bufs=2))
        wqkp = ctx.enter_context(tc.tile_pool(name="wqkp", bufs=3))
        wvhp = ctx.enter_context(tc.tile_pool(name="wvhp", bufs=2))
        rawp = ctx.enter_context(tc.tile_pool(name="rawp", bufs=3))
        m2p = ctx.enter_context(tc.tile_pool(name="m2p", bufs=4))
        ep = ctx.enter_context(tc.tile_pool(name="ep", bufs=4))
        stp = ctx.enter_context(tc.tile_pool(name="stp", bufs=2))
        big = ctx.enter_context(tc.tile_pool(name="big", bufs=3, space="PSUM"))
        small = ctx.enter_context(tc.tile_pool(name="small", bufs=2, space="PSUM"))

        # ---- resident tensors
        cosbt = const.tile([P, S], bf16, name="cosb_t")
        sinbt = const.tile([P, S], bf16, name="sinb_t")
        va = const.tile([P, 5 * NH * P], bf16, name="va")         # [j, head, 128]: v | ones | zeros
        vav = va.rearrange("p (j h c) -> p j h c", j=5, c=P)
        oT = const.tile([P, NKD * S], bf16, name="oT")
        obuf = const.tile([P, 16 * CH], bf16, name="obuf")        # per-group unnormalized o
        dbuf = const.tile([64, CH], bf16, name="dbuf")            # softmax denominators
        drecip = const.tile([64, CH], f32r, name="drecip")
        selc = const.tile([64, NH * HD], f32r, name="selc")       # row-select stationaries

        # ---- one-time setup
        nc.vector.memset(va[:, :], 0.0)
        nc.vector.memset(vav[:, :, :, HD:HD + 1], 1.0)
        nc.sync.dma_start(cosbt[:, :], cos_d[:, :])
        nc.sync.dma_start(sinbt[:, :], sin_d[:, :])
        nc.sync.dma_start(selc[:, :], sel_d[:, :])

        def load_x():
            xt = xtp.tile([P, NKD * S], bf16, tag="xt", name="xt")
            nc.sync.dma_start(xt.rearrange("p (kt s) -> p kt s", s=S),
                              xT_d.rearrange("(kt p) s -> p kt s", p=P))
            return xt

        def emit_qk_head(et, xt, roped, pq_state, step):
            """One slice of the next iteration's QK projection + RoPE.

            step 0: issue the weight DMA and allocate the PSUM accumulator.
            steps 1..5: emit a few of the 16 accumulation matmuls (these fill
            PE gaps left by the ScalarE exp of the surrounding attention).
            step 6: PSUM->SBUF copy (ScalarE) + the RoPE ops (DVE).
            """
            if step == 0:
                wt = wqkp.tile([P, NKD * P], bf16, tag="wt", name="wt")
                nc.sync.dma_start(wt.rearrange("p (kt c) -> p kt c", c=P),
                                  wqk_d[:, et * P:(et + 1) * P].rearrange("(kt p) c -> p kt c", p=P))
                pq = big.tile([P, 1024], f32, tag="big", name="pq")
                pq_state["wt"] = wt
                pq_state["pq"] = pq
                return
            wt, pq = pq_state["wt"], pq_state["pq"]
            if step <= 5:
                # kt chunks per step: (2,2,2,1,1) -> 8 kts over steps 1..5
                kts = {1: (0, 2), 2: (2, 4), 3: (4, 6), 4: (6, 7), 5: (7, 8)}[step]
                for kt in range(*kts):
                    w_r = wt[:, kt * P:(kt + 1) * P]
                    nc.tensor.matmul(pq[:, 0:CH], w_r, xt[:, kt * S:kt * S + CH],
                                     start=(kt == 0), stop=(kt == NKD - 1))
                    nc.tensor.matmul(pq[:, 512:512 + CH], w_r, xt[:, kt * S + CH:kt * S + S],
                                     start=(kt == 0), stop=(kt == NKD - 1))
                return
            raw = rawp.tile([P, S], bf16, name="raw")
            nc.scalar.activation(raw.rearrange("p (a c) -> p a c", c=CH),
                                 pq.rearrange("p (a c) -> p a c", c=512)[:, :, 0:CH],
                                 AF.Copy)
            # RoPE: t = raw*sin (sign pre-baked), rotate t by 32 partitions
            # within each 64-block (GpSimd 1-input copies - the only idle
            # engine), rsl = raw*cos + rot(t)
            t = m2p.tile([P, S], bf16, tag="m2", name="t")
            m2 = m2p.tile([P, S], bf16, tag="m2", name="m2")
            nc.vector.tensor_mul(t[:, :], raw[:, :], sinbt[:, :])
            for b0 in range(0, P, 64):
                nc.vector.tensor_copy(m2[b0 + 32:b0 + 64, :], t[b0:b0 + 32, :])
                nc.vector.tensor_copy(m2[b0:b0 + 32, :], t[b0 + 32:b0 + 64, :])
            rsl = roped[:, et * S:(et + 1) * S]
            nc.vector.tensor_mul(rsl, raw[:, :], cosbt[:, :])
            nc.vector.tensor_add(rsl, rsl, m2[:, :])

        def emit_v(xt):
            # 512-wide moving chunks (full PSUM bank per matmul); two s-tiles
            # share one [128,1024] accumulator tile.
            for half in range(2):
                wvt = wvhp.tile([P, NKD * 512], bf16, tag="wh", name="wvt")
                nc.sync.dma_start(wvt.rearrange("p (kt c) -> p kt c", c=512),
                                  wv_d[:, half * 512:(half + 1) * 512]
                                  .rearrange("(kt p) c -> p kt c", p=P))
                for sp in range(3):                     # s-tile pairs (0,1),(2,3),(4,)
                    pv = big.tile([P, 1024], f32, tag="big", name="pv")
                    nst = 2 if sp < 2 else 1
                    for kt in range(NKD):
                        for si in range(nst):
                            s0, sl = S_TILES[2 * sp + si]
                            nc.tensor.matmul(pv[0:sl, si * 512:si * 512 + 512],
                                             xt[:, kt * S + s0:kt * S + s0 + sl],
                                             wvt[:, kt * 512:(kt + 1) * 512],
                                             start=(kt == 0), stop=(kt == NKD - 1))
                    pvv = pv.rearrange("p (si h e) -> p si h e", si=2, e=HD)
                    va_r = va.rearrange("p (j h c) -> p j h c", j=5, c=P)
                    for si in range(nst):
                        st = 2 * sp + si
                        nc.vector.tensor_copy(
                            va_r[0:S_TILES[st][1], st, half * 8:half * 8 + 8, 0:HD],
                            pvv[0:S_TILES[st][1], si, :, :])

        def emit_epi_pair(g, h8):
            """Normalize one head of group g: broadcast 1/denom (selector
            matmul) and scale the two oT half-row blocks."""
            h = g * 8 + h8
            et = h // 2
            off = 64 * (h % 2)
            rw = big.tile([P, 1024], f32, tag="big", name="rw")
            for hf in range(2):
                gh = h8 * 2 + hf
                rr = rw[:, hf * 512:hf * 512 + CH]
                nc.tensor.matmul(rr[0:HD, :],
                                 selc[g * 32:g * 32 + 16, gh * HD:(gh + 1) * HD],
                                 drecip[g * 32:g * 32 + 16, :],
                                 start=True, stop=True)
                nc.vector.tensor_mul(
                    oT[off:off + 64, et * S + hf * CH:et * S + (hf + 1) * CH],
                    obuf[0:HD, gh * CH:(gh + 1) * CH], rr[0:HD, :])

        def emit_att(roped, qk_next):
            """Attention over all heads; qk_next=(xt_next, roped_next) or None.

            Group 0's normalization is interleaved into group 1's head loop
            (one head per head) so the PE never sits idle on the reciprocal
            chain; group 1's normalization trails and is covered by the next
            V projection emitted right after this.
            """
            for g in range(2):
                for h8 in range(8):
                    if g == 1:
                        emit_epi_pair(0, h8)
                    h = g * 8 + h8
                    et = h // 2
                    off = 64 * (h % 2)
                    qb = et * S
                    kb = (8 + et) * S
                    pq_state = {}
                    if qk_next is not None:
                        emit_qk_head(h, qk_next[0], qk_next[1], pq_state, 0)
                    po = [small.tile([P, CH], f32, tag="sm", name="ps_o") for _ in range(2)]
                    for j, (k0, kl) in enumerate(S_TILES):
                        sc = big.tile([P, 1024], f32, tag="big", name="sc")
                        stat = roped[off:off + 64, kb + k0:kb + k0 + kl]
                        for hf in range(2):
                            nc.tensor.matmul(sc[0:kl, hf * 512:hf * 512 + CH], stat,
                                             roped[off:off + 64, qb + hf * CH:qb + (hf + 1) * CH],
                                             start=True, stop=True)
                        if qk_next is not None:
                            emit_qk_head(h, qk_next[0], qk_next[1], pq_state, j + 1)
                        E = ep.tile([P, 2 * CH], bf16, name="E")
                        nc.scalar.activation(E.rearrange("p (a c) -> p a c", c=CH)[0:kl],
                                             sc.rearrange("p (a c) -> p a c", c=512)[0:kl, :, 0:CH],
                                             AF.Exp, scale=0.125)
                        for hf in range(2):
                            nc.tensor.matmul(po[hf][:, :], vav[0:kl, j:j + 1, h:h + 1, :],
                                             E[0:kl, hf * CH:(hf + 1) * CH],
                                             start=(j == 0), stop=(j == 4))
                    if qk_next is not None:
                        emit_qk_head(h, qk_next[0], qk_next[1], pq_state, 6)
                    for hf in range(2):
                        gh = h8 * 2 + hf
                        nc.vector.tensor_copy(obuf[0:HD + 1, gh * CH:(gh + 1) * CH],
                                              po[hf][0:HD + 1, :])
                # spread the 16 denominator segments (row 64 of obuf) across
                # partitions g*32..g*32+16 with one SBUF->SBUF DMA
                nc.sync.dma_start(dbuf[g * 32:g * 32 + 16, :],
                                  obuf[HD:HD + 1, :].rearrange("p (i c) -> p i c", c=CH))
                with nc.allow_low_precision(reason="f32r reciprocal, fp32 storage"):
                    nc.vector.reciprocal(drecip[g * 32:g * 32 + 16, :], dbuf[g * 32:g * 32 + 16, :])
            # group 1's normalization tail is emitted by the caller, after
            # the next V projection, so its reciprocal chain hides under the
            # V matmuls instead of stalling the PE behind pool FIFO order

        def emit_out():
            for half in range(2):
                wot = wvhp.tile([P, NKD * 512], bf16, tag="wh", name="wot")
                nc.sync.dma_start(wot.rearrange("p (kt c) -> p kt c", c=512),
                                  wo_d[:, half * 512:(half + 1) * 512]
                                  .rearrange("(kt p) c -> p kt c", p=P))
                for sp in range(3):
                    pw = big.tile([P, 1024], f32, tag="big", name="pw")
                    nst = 2 if sp < 2 else 1
                    for kt in range(NKD):
                        for si in range(nst):
                            s0, sl = S_TILES[2 * sp + si]
                            nc.tensor.matmul(pw[0:sl, si * 512:si * 512 + 512],
                                             oT[:, kt * S + s0:kt * S + s0 + sl],
                                             wot[:, kt * 512:(kt + 1) * 512],
                                             start=(kt == 0), stop=(kt == NKD - 1))
                    stg = stp.tile([P, 1024], f32, name="stg")
                    for si in range(nst):
                        s0, sl = S_TILES[2 * sp + si]
                        nc.scalar.activation(stg[0:sl, si * 512:(si + 1) * 512],
                                             pw[0:sl, si * 512:(si + 1) * 512], AF.Copy)
                        nc.sync.dma_start(out_d[s0:s0 + sl, half * 512:(half + 1) * 512],
                                          stg[0:sl, si * 512:(si + 1) * 512])

        # ---- prologue: x_0 + standalone QK_0 + V_0; x_1 prefetch
        xt_cur = load_x()
        roped_cur = ropedp.tile([P, 2 * NH * S], bf16, tag="roped", name="roped")
        for et in range(16):
            st_qk = {}
            emit_qk_head(et, xt_cur, roped_cur, st_qk, 0)
            for sp in range(1, 6):
                emit_qk_head(et, xt_cur, roped_cur, st_qk, sp)
            emit_qk_head(et, xt_cur, roped_cur, st_qk, 6)
        emit_v(xt_cur)
        xt_next = load_x() if repeat > 1 else None

        for _rep in range(repeat):
            if _rep + 1 < repeat:
                roped_next = ropedp.tile([P, 2 * NH * S], bf16, tag="roped", name="roped")
                emit_att(roped_cur, (xt_next, roped_next))
                # prefetch x for iteration _rep+2's interleaved QK: the DMA
                # hides under the V/OUT work emitted below
                xt_follow = load_x() if _rep + 2 < repeat else None
                # next iteration's V projection: its PE work covers this
                # iteration's trailing normalization + the out-proj DMAs
                emit_v(xt_next)
                for h8 in range(8):
                    emit_epi_pair(1, h8)
                xt_cur, roped_cur = xt_next, roped_next
                xt_next = xt_follow
            else:
                emit_att(roped_cur, None)
                for h8 in range(8):
                    emit_epi_pair(1, h8)
            emit_out()
    nc.compile()
    return nc


def _prep_inputs(x, w_qkv, w_out):
    import ml_dtypes
    bf = ml_dtypes.bfloat16
    x = np.asarray(x, dtype=np.float32)
    w_qkv = np.asarray(w_qkv, dtype=np.float32)
    w_out = np.asarray(w_out, dtype=np.float32)
    xr = x.reshape(BT, S, D)
    perm = _head_perm()
    wq = w_qkv[0:D][perm]
    wk = w_qkv[D:2 * D][perm]
    wqkT = np.ascontiguousarray(np.concatenate([wq, wk], axis=0).T).astype(bf)
    wvT = np.ascontiguousarray(w_qkv[2 * D:3 * D].T).astype(bf)
    woT = np.ascontiguousarray(w_out.T).astype(bf)
    cosb, sinb = _rope_tables()
    cosb = cosb.astype(bf)
    sinb = sinb.astype(bf)
    selc = np.zeros((64, NH * HD), dtype=np.float32)
    for j in range(16):
        selc[j, j * HD:(j + 1) * HD] = 1.0
        selc[32 + j, j * HD:(j + 1) * HD] = 1.0
    in_maps = []
    for i in range(NCORES):
        in_maps.append({
            "xT": np.ascontiguousarray(xr[i].T).astype(bf),
            "wqkT": wqkT, "wvT": wvT, "woT": woT,
            "cosb": cosb, "sinb": sinb, "selc": selc,
        })
    return in_maps


def get_nc(repeat=1):
    key = f"nc{repeat}"
    if key not in _CACHE:
        _CACHE[key] = _build_nc(repeat)
    return _CACHE[key]


def kernel(x, w_qkv, w_out, b_out):
    from concourse import bass_utils
    nc = get_nc()
    in_maps = _prep_inputs(x, w_qkv, w_out)
    res = bass_utils.run_bass_kernel_spmd(nc, in_maps, core_ids=list(range(NCORES)))
    out = np.stack([res.results[i]["out"] for i in range(NCORES)], axis=0)
    out = out + np.asarray(b_out, dtype=np.float32)[None, None, :]
    return np.ascontiguousarray(out.reshape(B, T, HH, WW, D).astype(np.float32))
